# revision 1
# baseline (speedup 1.0000x reference)
import os
import sys

for _p in ("/opt/trn_rl_repo", "/root/.axon_site/_ro/trn_rl_repo"):
    if os.path.isdir(_p) and _p not in sys.path:
        sys.path.insert(0, _p)

import numpy as np

HEADS, D = 12, 64
WINDOW, SHIFT = 16, 1
SCALE = D ** -0.5
B, N, DIM = 2, 2049, 768
INNER = HEADS * D  # 768
TAUG = 258  # CLS slot + tok1/dummy slot + 256 block tokens
NCORES = 8
KT = DIM // 128  # 6

# global token ranges owned by each core (block attention); all starts == 2 mod 16
STARTS = [2, 258, 514, 770, 1026, 1282, 1538, 1794]
ENDS = [258, 514, 770, 1026, 1282, 1538, 1794, 2049]

_NC_CACHE = {}


def _build_nc():
    import concourse.bass as bass
    import concourse.bacc as bacc
    import concourse.mybir as mybir
    import concourse.tile as tile

    f32 = mybir.dt.float32
    Exp = mybir.ActivationFunctionType.Exp

    nc = bacc.Bacc(None, target_bir_lowering=False)

    x_ext = nc.declare_dram_parameter("xa", (B, TAUG, DIM), f32, isOutput=False)
    wqkv_ext = nc.declare_dram_parameter("w_qkv", (DIM, 3 * INNER), f32, isOutput=False)
    wout_ext = nc.declare_dram_parameter("w_out", (INNER, DIM), f32, isOutput=False)
    bout_ext = nc.declare_dram_parameter("b_out", (128, DIM), f32, isOutput=False)
    mask_ext = nc.declare_dram_parameter("masks", (2, 128, 128), f32, isOutput=False)
    id_ext = nc.declare_dram_parameter("ident", (128, 128), f32, isOutput=False)
    out_ext = nc.declare_dram_parameter("out_tokens", (B, TAUG, DIM), f32, isOutput=True)
    clsp_ext = nc.declare_dram_parameter("cls_part", (B, INNER + HEADS), f32, isOutput=True)
    clse_ext = nc.declare_dram_parameter("cls_extra", (B, INNER + HEADS), f32, isOutput=True)

    with tile.TileContext(nc) as tc:
        with (
            tc.tile_pool(name="wpool", bufs=1) as wp,
            tc.tile_pool(name="fpool", bufs=1) as fp,
            tc.tile_pool(name="inpool", bufs=2) as ip,
            tc.tile_pool(name="spool", bufs=3) as sp,
            tc.tile_pool(name="psA", bufs=2, space="PSUM") as psA,  # big [128,512]
            tc.tile_pool(name="psB", bufs=2, space="PSUM") as psB,  # av [64,128]
            tc.tile_pool(name="psC", bufs=2, space="PSUM") as psC,  # small
            tc.tile_pool(name="psD", bufs=1, space="PSUM") as psD,  # cls accum [1,64]/[1,1]
        ):
            # ---- constants / weights ----
            ident = wp.tile([128, 128], f32, tag="ident")
            nc.sync.dma_start(ident[:], id_ext[:])
            mask_t = []
            for s in range(2):
                m = wp.tile([128, 128], f32, tag=f"mask{s}")
                nc.sync.dma_start(m[:], mask_ext[s])
                mask_t.append(m)
            ones = wp.tile([128, 1], f32, tag="ones")
            nc.vector.memset(ones[:], 1.0)
            bias_full = wp.tile([128, DIM], f32, tag="bias_full")
            nc.sync.dma_start(bias_full[:], bout_ext[:])
            w_tiles = []
            for k in range(KT):
                t = wp.tile([128, 3 * INNER], f32, tag=f"wqkv{k}")
                nc.sync.dma_start(t[:], wqkv_ext[k * 128:(k + 1) * 128, :])
                w_tiles.append(t)
            wo_t = []
            for k in range(KT):
                t = wp.tile([128, DIM], f32, tag=f"wo{k}")
                nc.sync.dma_start(t[:], wout_ext[k * 128:(k + 1) * 128, :])
                wo_t.append(t)

            for b in range(B):
                # ---- load x and transpose to feature-major xT [768, TAUG] ----
                xT = [fp.tile([128, TAUG], f32, tag=f"xT{k}", name=f"xT{k}") for k in range(KT)]
                for tt in range(3):
                    rows = 128 if tt < 2 else TAUG - 256
                    xt = ip.tile([128, DIM], f32, tag="xt")
                    nc.sync.dma_start(xt[:rows, :], x_ext[b, tt * 128:tt * 128 + rows, :])
                    for k in range(KT):
                        pt = psA.tile([128, 128], f32, tag="big")
                        nc.tensor.transpose(pt[:, :rows], xt[:rows, k * 128:(k + 1) * 128], ident[:rows, :rows])
                        nc.vector.tensor_copy(xT[k][:, tt * 128:tt * 128 + rows], pt[:, :rows])

                # ---- per-head qT, kT (feature-major, 64-part tiles) ----
                qT, kT = [], []
                for h in range(HEADS):
                    pq = psA.tile([64, TAUG], f32, tag="big")
                    for k in range(KT):
                        nc.tensor.matmul(pq[:], w_tiles[k][:, h * 64:(h + 1) * 64], xT[k][:],
                                         start=(k == 0), stop=(k == KT - 1))
                    q = fp.tile([64, TAUG], f32, tag=f"qT{h}")
                    nc.vector.tensor_copy(q[:], pq[:])
                    qT.append(q)
                    pk = psA.tile([64, TAUG], f32, tag="big")
                    for k in range(KT):
                        nc.tensor.matmul(pk[:], w_tiles[k][:, INNER + h * 64:INNER + (h + 1) * 64], xT[k][:],
                                         start=(k == 0), stop=(k == KT - 1))
                    kk = fp.tile([64, TAUG], f32, tag=f"kT{h}")
                    nc.vector.tensor_copy(kk[:], pk[:])
                    kT.append(kk)

                # ---- v token-major: two 128-token stripes (p=2..130, 130..258) + p=0:2 ----
                vs = []
                for s in range(2):
                    vt = fp.tile([128, INNER], f32, tag=f"v{s}")
                    for half, (c0, cw) in enumerate([(0, 512), (512, 256)]):
                        pv = psA.tile([128, cw], f32, tag="big")
                        for k in range(KT):
                            nc.tensor.matmul(pv[:], xT[k][:, 2 + s * 128: 2 + (s + 1) * 128],
                                             w_tiles[k][:, 2 * INNER + c0: 2 * INNER + c0 + cw],
                                             start=(k == 0), stop=(k == KT - 1))
                        nc.vector.tensor_copy(vt[:, c0:c0 + cw], pv[:])
                    vs.append(vt)
                vc = fp.tile([2, INNER], f32, tag="vc")
                for half, (c0, cw) in enumerate([(0, 512), (512, 256)]):
                    pvc = psC.tile([2, 512], f32, tag="small")
                    for k in range(KT):
                        nc.tensor.matmul(pvc[:, :cw], xT[k][:, 0:2],
                                         w_tiles[k][:, 2 * INNER + c0: 2 * INNER + c0 + cw],
                                         start=(k == 0), stop=(k == KT - 1))
                    nc.vector.tensor_copy(vc[:, c0:c0 + cw], pvc[:, :cw])

                # ---- attention (token-major AV; per-query recip via tensor_scalar) ----
                att_s = [ip.tile([128, INNER], f32, tag=f"att{s}", name=f"att{s}") for s in range(2)]
                att1row = ip.tile([1, INNER], f32, tag="att1row")
                clsacc = sp.tile([1, INNER + HEADS], f32, tag="clsacc")
                clsext = sp.tile([1, INNER + HEADS], f32, tag="clsext")
                for h in range(HEADS):
                    # CLS-as-key row: exp(q_p . k_cls) for all p
                    pcr = psC.tile([1, TAUG], f32, tag="small")
                    nc.tensor.matmul(pcr[:], kT[h][:, 0:1], qT[h][:], start=True, stop=True)
                    ecr = sp.tile([1, TAUG], f32, tag="ecr")
                    nc.scalar.activation(ecr[:], pcr[:], Exp, scale=SCALE)

                    # token-1 special (keys p=0,1; query p=1)
                    pt1 = psC.tile([2, 1], f32, tag="small")
                    nc.tensor.matmul(pt1[:], kT[h][:, 0:2], qT[h][:, 1:2], start=True, stop=True)
                    et1 = sp.tile([2, 1], f32, tag="et1")
                    nc.scalar.activation(et1[:], pt1[:], Exp, scale=SCALE)
                    pav1 = psC.tile([1, 64], f32, tag="small")
                    nc.tensor.matmul(pav1[:], et1[:], vc[:, h * 64:(h + 1) * 64], start=True, stop=True)
                    ps1 = psC.tile([1, 1], f32, tag="small")
                    nc.tensor.matmul(ps1[:], et1[:], ones[0:2, :], start=True, stop=True)
                    r1 = sp.tile([1, 1], f32, tag="r1")
                    nc.vector.reciprocal(r1[:], ps1[:])
                    nc.vector.tensor_scalar_mul(att1row[:, h * 64:(h + 1) * 64], pav1[:], r1[:])

                    # CLS-query partials over stripes (lhsT = exp column; v moving)
                    po_cls = psD.tile([1, 64], f32, tag="pocls")
                    ps_cls = psD.tile([1, 1], f32, tag="pscls")
                    for s in range(2):
                        q0 = 2 + s * 128
                        pcc = psC.tile([128, 1], f32, tag="small")
                        nc.tensor.matmul(pcc[:], kT[h][:, q0:q0 + 128], qT[h][:, 0:1], start=True, stop=True)
                        ecc = sp.tile([128, 1], f32, tag="ecc")
                        nc.scalar.activation(ecc[:], pcc[:], Exp, scale=SCALE)
                        nc.tensor.matmul(po_cls[:], ecc[:], vs[s][:, h * 64:(h + 1) * 64],
                                         start=(s == 0), stop=(s == 1))
                        nc.tensor.matmul(ps_cls[:], ecc[:], ones[:, 0:1], start=(s == 0), stop=(s == 1))
                    nc.vector.tensor_copy(clsacc[:, h * 64:(h + 1) * 64], po_cls[:])
                    nc.vector.tensor_copy(clsacc[:, INNER + h:INNER + h + 1], ps_cls[:])

                    # extra piece (keys p=0,1 for CLS query) — host uses core 0's only
                    pce = psC.tile([2, 1], f32, tag="small")
                    nc.tensor.matmul(pce[:], kT[h][:, 0:2], qT[h][:, 0:1], start=True, stop=True)
                    ece = sp.tile([2, 1], f32, tag="ece")
                    nc.scalar.activation(ece[:], pce[:], Exp, scale=SCALE)
                    pe_o = psD.tile([1, 64], f32, tag="pocls")
                    pe_s = psD.tile([1, 1], f32, tag="pscls")
                    nc.tensor.matmul(pe_o[:], ece[:], vc[:, h * 64:(h + 1) * 64], start=True, stop=True)
                    nc.tensor.matmul(pe_s[:], ece[:], ones[0:2, 0:1], start=True, stop=True)
                    nc.vector.tensor_copy(clsext[:, h * 64:(h + 1) * 64], pe_o[:])
                    nc.vector.tensor_copy(clsext[:, INNER + h:INNER + h + 1], pe_s[:])

                    # block-diagonal stripes
                    for s in range(2):
                        q0 = 2 + s * 128
                        pst = psA.tile([128, 128], f32, tag="big")
                        nc.tensor.matmul(pst[:], kT[h][:, q0:q0 + 128], qT[h][:, q0:q0 + 128],
                                         start=True, stop=True)
                        prob = sp.tile([128, 128], f32, tag="prob")
                        nc.scalar.activation(prob[:], pst[:], Exp, scale=SCALE)
                        nc.vector.tensor_mul(prob[:], prob[:], mask_t[s][:])
                        pav = psB.tile([128, 64], f32, tag="av")
                        nc.tensor.matmul(pav[:], prob[:], vs[s][:, h * 64:(h + 1) * 64], start=True, stop=False)
                        nc.tensor.matmul(pav[:], ecr[:, q0:q0 + 128], vc[0:1, h * 64:(h + 1) * 64],
                                         start=False, stop=True)
                        psums = psC.tile([128, 1], f32, tag="small")
                        nc.tensor.matmul(psums[:], prob[:], ones[:, 0:1], start=True, stop=False)
                        nc.tensor.matmul(psums[:], ecr[:, q0:q0 + 128], ones[0:1, 0:1], start=False, stop=True)
                        rec = sp.tile([128, 1], f32, tag="rec")
                        nc.vector.reciprocal(rec[:], psums[:])
                        nc.vector.tensor_scalar_mul(att_s[s][:, h * 64:(h + 1) * 64], pav[:], rec[:])

                # ---- transpose attention output to feature-major aTfm [768, TAUG] ----
                aTfm = [fp.tile([128, TAUG], f32, tag=f"aTfm{i}", name=f"aTfm{i}") for i in range(KT)]
                for i in range(KT):
                    for s in range(2):
                        q0 = 2 + s * 128
                        pt = psA.tile([128, 128], f32, tag="big", name="ptr")
                        nc.tensor.transpose(pt[:], att_s[s][:, i * 128:(i + 1) * 128], ident[:])
                        nc.vector.tensor_copy(aTfm[i][:, q0:q0 + 128], pt[:])
                    pt1r = psC.tile([128, 1], f32, tag="small")
                    nc.tensor.transpose(pt1r[:], att1row[:, i * 128:(i + 1) * 128], ident[0:1, 0:1])
                    nc.vector.tensor_copy(aTfm[i][:, 1:2], pt1r[:])

                # ---- write CLS partials ----
                nc.sync.dma_start(clsp_ext[b:b + 1, :], clsacc[:])
                nc.sync.dma_start(clse_ext[b:b + 1, :], clsext[:])

                # ---- out projection: out[p, :] = aTfm[:, p].T @ w_out + b ----
                for tt in range(3):
                    c0 = tt * 128
                    cw = 128 if tt < 2 else TAUG - 256
                    ot = ip.tile([128, DIM], f32, tag="ot")
                    for half, (f0, fw) in enumerate([(0, 512), (512, 256)]):
                        po = psA.tile([128, fw], f32, tag="big")
                        for i in range(KT):
                            nc.tensor.matmul(po[:cw, :], aTfm[i][:, c0:c0 + cw], wo_t[i][:, f0:f0 + fw],
                                             start=(i == 0), stop=(i == KT - 1))
                        nc.vector.tensor_add(ot[:cw, f0:f0 + fw], po[:cw, :], bias_full[:cw, f0:f0 + fw])
                    nc.sync.dma_start(out_ext[b, c0:c0 + cw, :], ot[:cw, :])

    nc.compile()
    return nc


def _get_nc():
    if "nc" not in _NC_CACHE:
        _NC_CACHE["nc"] = _build_nc()
    return _NC_CACHE["nc"]


def _make_masks(core):
    start = STARTS[core]
    masks = np.zeros((2, 128, 128), dtype=np.float32)
    for s in range(2):
        g = start + s * 128 + np.arange(128)  # global tokens for p = 2+128s .. +128
        real = g < 2049
        blk = (g - 2) // 16
        same = (blk[:, None] == blk[None, :]) & real[:, None] & real[None, :]
        masks[s] = same.astype(np.float32)
    return masks


def kernel(x, w_qkv, w_out, b_out):
    x = np.asarray(x, dtype=np.float32)
    w_qkv = np.asarray(w_qkv, dtype=np.float32)
    w_out = np.asarray(w_out, dtype=np.float32)
    b_out = np.asarray(b_out, dtype=np.float32)

    ident = np.eye(128, dtype=np.float32)
    in_maps = []
    for c in range(NCORES):
        xa = np.zeros((B, TAUG, DIM), dtype=np.float32)
        xa[:, 0, :] = x[:, 0, :]
        if c == 0:
            xa[:, 1, :] = x[:, 1, :]
        L = ENDS[c] - STARTS[c]
        xa[:, 2:2 + L, :] = x[:, STARTS[c]:ENDS[c], :]
        in_maps.append({
            "xa": xa,
            "w_qkv": w_qkv,
            "w_out": w_out,
            "b_out": np.tile(b_out.reshape(1, DIM), (128, 1)),
            "masks": _make_masks(c),
            "ident": ident,
        })

    from concourse.bass_utils import run_bass_kernel_spmd

    nc = _get_nc()
    res = run_bass_kernel_spmd(nc, in_maps, core_ids=list(range(NCORES))).results

    out = np.empty((B, N, DIM), dtype=np.float32)
    for c in range(NCORES):
        L = ENDS[c] - STARTS[c]
        out[:, STARTS[c]:ENDS[c], :] = res[c]["out_tokens"][:, 2:2 + L, :]
    out[:, 1, :] = res[0]["out_tokens"][:, 1, :]

    # CLS row from partial softmax stats
    for b in range(B):
        o = res[0]["cls_extra"][b].astype(np.float64).copy()
        for c in range(NCORES):
            o = o + res[c]["cls_part"][b].astype(np.float64)
            # padding keys on core c contributed exp(0)=1 to each head's sum
            o[INNER:] -= 256 - (ENDS[c] - STARTS[c])
        ov = o[:INNER].reshape(HEADS, 64)
        s = o[INNER:]  # [HEADS]
        flat = (ov / s[:, None]).reshape(INNER)  # f = h*64 + d
        out[b, 0, :] = (flat @ w_out + b_out).astype(np.float32)
    return out



# revision 6
# speedup vs baseline: 3.3654x; 3.3654x over previous
import os
import sys

for _p in ("/opt/trn_rl_repo", "/root/.axon_site/_ro/trn_rl_repo"):
    if os.path.isdir(_p) and _p not in sys.path:
        sys.path.insert(0, _p)

import numpy as np
import ml_dtypes

BF16 = ml_dtypes.bfloat16

HEADS, D = 12, 64
WINDOW, SHIFT = 16, 1
SCALE = D ** -0.5
B, N, DIM = 2, 2049, 768
INNER = HEADS * D  # 768
TAUG = 258  # CLS slot + tok1/dummy slot + 256 block tokens
NCORES = 8
KT = DIM // 128  # 6

# global token ranges owned by each core (block attention); all starts == 2 mod 16
STARTS = [2, 258, 514, 770, 1026, 1282, 1538, 1794]
ENDS = [258, 514, 770, 1026, 1282, 1538, 1794, 2049]

_NC_CACHE = {}


def _build_nc():
    import concourse.bass as bass
    import concourse.bacc as bacc
    import concourse.mybir as mybir
    import concourse.tile as tile

    f32 = mybir.dt.float32
    bf16 = mybir.dt.bfloat16
    Exp = mybir.ActivationFunctionType.Exp
    Copy = mybir.ActivationFunctionType.Copy

    nc = bacc.Bacc(None, target_bir_lowering=False)

    xT_ext = nc.declare_dram_parameter("xaT", (B, KT, 128, TAUG), bf16, isOutput=False)
    wqkv_ext = nc.declare_dram_parameter("w_qkv", (DIM, 3 * INNER), bf16, isOutput=False)
    wout_ext = nc.declare_dram_parameter("w_out", (INNER, DIM), bf16, isOutput=False)
    bout_ext = nc.declare_dram_parameter("b_out", (128, DIM), f32, isOutput=False)
    mask_ext = nc.declare_dram_parameter("masks", (2, 128, 128), bf16, isOutput=False)
    id_ext = nc.declare_dram_parameter("ident", (128, 128), bf16, isOutput=False)
    out_ext = nc.declare_dram_parameter("out_tokens", (B, 256, DIM), f32, isOutput=True)
    cls_ext = nc.declare_dram_parameter("cls_part", (B, HEADS, 769), f32, isOutput=True)
    t1x_ext = nc.declare_dram_parameter("t1x_part", (B, 2 * HEADS, 769), f32, isOutput=True)

    with tile.TileContext(nc) as tc:
        with (
            tc.tile_pool(name="wpool", bufs=1) as wp,
            tc.tile_pool(name="fpool", bufs=2) as fp,
            tc.tile_pool(name="spool", bufs=3) as sp,
            tc.tile_pool(name="psA", bufs=2, space="PSUM") as psA,  # big [128,512]
            tc.tile_pool(name="psP", bufs=2, space="PSUM") as psP,  # pst + transposes
            tc.tile_pool(name="psS", bufs=2, space="PSUM") as psS,  # hps per-head + cls per-batch
        ):
            # ---- constants / weights ----
            ident = wp.tile([128, 128], bf16, tag="ident")
            nc.sync.dma_start(ident[:], id_ext[:])
            mask_t = []
            for s in range(2):
                m = wp.tile([128, 128], bf16, tag=f"mask{s}")
                nc.sync.dma_start(m[:], mask_ext[s])
                mask_t.append(m)
            ones = wp.tile([128, 1], bf16, tag="ones")
            nc.vector.memset(ones[:], 1.0)
            bias_full = wp.tile([128, DIM], f32, tag="bias_full")
            nc.sync.dma_start(bias_full[:], bout_ext[:])
            w_tiles = []
            for k in range(KT):
                t = wp.tile([128, 3 * INNER], bf16, tag=f"wqkv{k}")
                nc.sync.dma_start(t[:], wqkv_ext[k * 128:(k + 1) * 128, :])
                w_tiles.append(t)
            wo_t = []
            for k in range(KT):
                t = wp.tile([128, DIM], bf16, tag=f"wo{k}")
                nc.sync.dma_start(t[:], wout_ext[k * 128:(k + 1) * 128, :])
                wo_t.append(t)

            for b in range(B):
                # ---- x already feature-major from host: xT[k] = [128 dims, TAUG toks]
                xT = []
                for k in range(KT):
                    t = fp.tile([128, TAUG], bf16, tag=f"xT{k}", name=f"xT{k}")
                    nc.sync.dma_start(t[:], xT_ext[b, k])
                    xT.append(t)

                # ---- Q, K feature-major in head pairs: qT[g] = heads {2g, 2g+1}
                qT, kTt = [], []
                for base, pref, lst in ((0, "q", qT), (INNER, "k", kTt)):
                    for g in range(HEADS // 2):
                        ps = psA.tile([128, 512], f32, tag="big")
                        for k in range(KT):
                            nc.tensor.matmul(ps[:, 0:TAUG],
                                             w_tiles[k][:, base + 128 * g: base + 128 * (g + 1)],
                                             xT[k][:], start=(k == 0), stop=(k == KT - 1))
                        t = fp.tile([128, TAUG], bf16, tag=f"{pref}T{g}", name=f"{pref}T{g}")
                        nc.scalar.activation(t[:], ps[:, 0:TAUG], Copy)
                        lst.append(t)

                # ---- V token-major stripes [128 toks, 768] + ones col at 768
                vs = []
                for ti in range(2):
                    vt = fp.tile([128, INNER + 1], bf16, tag=f"v{ti}", name=f"v{ti}")
                    for c0, cw in ((0, 512), (512, 256)):
                        pv = psA.tile([128, 512], f32, tag="big")
                        for k in range(KT):
                            nc.tensor.matmul(pv[:, 0:cw],
                                             xT[k][:, 2 + 128 * ti: 2 + 128 * (ti + 1)],
                                             w_tiles[k][:, 2 * INNER + c0: 2 * INNER + c0 + cw],
                                             start=(k == 0), stop=(k == KT - 1))
                        nc.scalar.activation(vt[:, c0:c0 + cw], pv[:, 0:cw], Copy)
                    nc.vector.memset(vt[:, INNER:INNER + 1], 1.0)
                    vs.append(vt)
                vc = fp.tile([2, INNER + 1], bf16, tag="vc")
                for c0, cw in ((0, 512), (512, 256)):
                    pvc = psA.tile([128, 512], f32, tag="big")
                    for k in range(KT):
                        nc.tensor.matmul(pvc[0:2, 0:cw], xT[k][:, 0:2],
                                         w_tiles[k][:, 2 * INNER + c0: 2 * INNER + c0 + cw],
                                         start=(k == 0), stop=(k == KT - 1))
                    nc.scalar.activation(vc[:, c0:c0 + cw], pvc[0:2, 0:cw], Copy)
                nc.vector.memset(vc[:, INNER:INNER + 1], 1.0)

                # ---- attention ----
                # cls bank-tile per batch: cols 0:12 ecc_s0, 12:24 ecc_s1, 24:48 t1d
                att_s = [fp.tile([128, INNER], bf16, tag=f"att{s}", name=f"att{s}") for s in range(2)]
                clsp = psS.tile([128, 512], f32, tag="cls")
                for h in range(HEADS):
                    g, p0 = h // 2, 64 * (h % 2)
                    kk, qq = kTt[g], qT[g]
                    # per-head bank-tile: cols 0:258 pcr, 258:322 pav0, 322:323 sum0,
                    # 323:387 pav1, 387:388 sum1
                    hps = psS.tile([128, 512], f32, tag="hps")
                    # CLS-as-key row: exp(k_cls . q_p) for all p
                    nc.tensor.matmul(hps[0:1, 0:TAUG], kk[p0:p0 + 64, 0:1], qq[p0:p0 + 64, :],
                                     start=True, stop=True, skip_group_check=True)
                    ecr = sp.tile([1, TAUG], bf16, tag="ecr")
                    nc.scalar.activation(ecr[:], hps[0:1, 0:TAUG], Exp, scale=SCALE)
                    # CLS-query dots vs stripe keys -> column h of ecc
                    for s in range(2):
                        q0 = 2 + 128 * s
                        nc.tensor.matmul(clsp[:, 12 * s + h:12 * s + h + 1],
                                         kk[p0:p0 + 64, q0:q0 + 128],
                                         qq[p0:p0 + 64, 0:1], start=True, stop=True,
                                         skip_group_check=True)
                    # keys {CLS,tok1} x queries {CLS,tok1} -> cols 24+2h:24+2h+2
                    nc.tensor.matmul(clsp[0:2, 24 + 2 * h:24 + 2 * h + 2], kk[p0:p0 + 64, 0:2],
                                     qq[p0:p0 + 64, 0:2], start=True, stop=True,
                                     skip_group_check=True)
                    # block-diagonal stripes
                    for s in range(2):
                        q0 = 2 + 128 * s
                        pc = TAUG + 65 * s  # pav col base
                        pst = psP.tile([128, 128], f32, tag="pq")
                        nc.tensor.matmul(pst[:], kk[p0:p0 + 64, q0:q0 + 128],
                                         qq[p0:p0 + 64, q0:q0 + 128], start=True, stop=True)
                        prob = sp.tile([128, 128], bf16, tag="prob")
                        nc.scalar.activation(prob[:], pst[:], Exp, scale=SCALE)
                        nc.vector.tensor_mul(prob[:], prob[:], mask_t[s][:])
                        nc.tensor.matmul(hps[:, pc:pc + 64], prob[:], vs[s][:, 64 * h:64 * h + 64],
                                         start=True, stop=False, skip_group_check=True)
                        nc.tensor.matmul(hps[:, pc:pc + 64], ecr[:, q0:q0 + 128],
                                         vc[0:1, 64 * h:64 * h + 64],
                                         start=False, stop=True, skip_group_check=True)
                        nc.tensor.matmul(hps[:, pc + 64:pc + 65], prob[:], ones[:, 0:1],
                                         start=True, stop=False, skip_group_check=True)
                        nc.tensor.matmul(hps[:, pc + 64:pc + 65], ecr[:, q0:q0 + 128],
                                         ones[0:1, 0:1],
                                         start=False, stop=True, skip_group_check=True)
                        rec = sp.tile([128, 1], f32, tag="rec")
                        nc.vector.reciprocal(rec[:], hps[:, pc + 64:pc + 65])
                        nc.scalar.activation(att_s[s][:, 64 * h:64 * h + 64],
                                             hps[:, pc:pc + 64], Copy, scale=rec[:, 0:1])

                # ---- CLS-query partials: [12, 769] = ECC^T @ [V | 1], summed over stripes
                eccs = []
                for s in range(2):
                    E = sp.tile([128, HEADS], bf16, tag="ECC", name="E")
                    nc.scalar.activation(E[:], clsp[:, 12 * s:12 * (s + 1)], Exp, scale=SCALE)
                    eccs.append(E)
                clsA = psS.tile([128, 512], f32, tag="hps")
                clsB = psS.tile([128, 512], f32, tag="hps")
                for s in range(2):
                    nc.tensor.matmul(clsA[0:HEADS, :], eccs[s][:], vs[s][:, 0:512],
                                     start=(s == 0), stop=(s == 1), skip_group_check=True)
                    nc.tensor.matmul(clsB[0:HEADS, 0:257], eccs[s][:], vs[s][:, 512:769],
                                     start=(s == 0), stop=(s == 1), skip_group_check=True)
                cls_sb = sp.tile([HEADS, 769], f32, tag="clssb")
                nc.vector.tensor_copy(cls_sb[:, 0:512], clsA[0:HEADS, :])
                nc.vector.tensor_copy(cls_sb[:, 512:769], clsB[0:HEADS, 0:257])
                nc.sync.dma_start(cls_ext[b], cls_sb[:])

                # ---- tok1 + CLS-extra: [24, 769] = ET1^T @ [vc | 1]
                ET1 = sp.tile([2, 2 * HEADS], bf16, tag="ET1")
                nc.scalar.activation(ET1[:], clsp[0:2, 24:24 + 2 * HEADS], Exp, scale=SCALE)
                t1A = psS.tile([128, 512], f32, tag="hps")
                t1B = psS.tile([128, 512], f32, tag="hps")
                nc.tensor.matmul(t1A[0:24, :], ET1[:], vc[0:2, 0:512], start=True, stop=True,
                                 skip_group_check=True)
                nc.tensor.matmul(t1B[0:24, 0:257], ET1[:], vc[0:2, 512:769], start=True,
                                 stop=True, skip_group_check=True)
                t1_sb = sp.tile([24, 769], f32, tag="t1sb")
                nc.vector.tensor_copy(t1_sb[:, 0:512], t1A[0:24, :])
                nc.vector.tensor_copy(t1_sb[:, 512:769], t1B[0:24, 0:257])
                nc.sync.dma_start(t1x_ext[b], t1_sb[:])

                # ---- transpose attention output to feature-major [768, 256]
                aT = [fp.tile([128, 256], bf16, tag=f"aT{i}", name=f"aT{i}") for i in range(KT)]
                for i in range(KT):
                    for s in range(2):
                        pt = psP.tile([128, 128], bf16, tag="pq", name="pt")
                        nc.tensor.transpose(pt[:], att_s[s][:, 128 * i:128 * (i + 1)], ident[:])
                        nc.vector.tensor_copy(aT[i][:, 128 * s:128 * (s + 1)], pt[:])

                # ---- out projection (256 block tokens only)
                for ti in range(2):
                    ot = fp.tile([128, DIM], f32, tag="ot")
                    for c0, cw in ((0, 512), (512, 256)):
                        po = psA.tile([128, 512], f32, tag="big")
                        for i in range(KT):
                            nc.tensor.matmul(po[:, 0:cw], aT[i][:, 128 * ti:128 * (ti + 1)],
                                             wo_t[i][:, c0:c0 + cw],
                                             start=(i == 0), stop=(i == KT - 1))
                        nc.vector.tensor_add(ot[:, c0:c0 + cw], po[:, 0:cw], bias_full[:, c0:c0 + cw])
                    nc.sync.dma_start(out_ext[b, 128 * ti:128 * (ti + 1), :], ot[:])

    nc.compile()
    return nc


def _get_nc():
    if "nc" not in _NC_CACHE:
        _NC_CACHE["nc"] = _build_nc()
    return _NC_CACHE["nc"]


def _make_masks(core):
    start = STARTS[core]
    masks = np.zeros((2, 128, 128), dtype=np.float32)
    for s in range(2):
        g = start + s * 128 + np.arange(128)  # global tokens for p = 2+128s .. +128
        real = g < 2049
        blk = (g - 2) // 16
        same = (blk[:, None] == blk[None, :]) & real[:, None] & real[None, :]
        masks[s] = same.astype(np.float32)
    return masks.astype(BF16)


def _make_in_maps(x, w_qkv, w_out, b_out):
    x = np.asarray(x, dtype=np.float32)
    w_qkv_b = np.asarray(w_qkv, dtype=np.float32).astype(BF16)
    w_out_b = np.asarray(w_out, dtype=np.float32).astype(BF16)
    b_out = np.asarray(b_out, dtype=np.float32)

    ident = np.eye(128, dtype=BF16)
    bias_tiled = np.tile(b_out.reshape(1, DIM), (128, 1)).astype(np.float32)
    in_maps = []
    for c in range(NCORES):
        xa = np.zeros((B, TAUG, DIM), dtype=np.float32)
        xa[:, 0, :] = x[:, 0, :]
        if c == 0:
            xa[:, 1, :] = x[:, 1, :]
        L = ENDS[c] - STARTS[c]
        xa[:, 2:2 + L, :] = x[:, STARTS[c]:ENDS[c], :]
        xaT = xa.transpose(0, 2, 1).reshape(B, KT, 128, TAUG).astype(BF16)
        in_maps.append({
            "xaT": xaT,
            "w_qkv": w_qkv_b,
            "w_out": w_out_b,
            "b_out": bias_tiled,
            "masks": _make_masks(c),
            "ident": ident,
        })
    return in_maps


def kernel(x, w_qkv, w_out, b_out):
    w_out_f = np.asarray(w_out, dtype=np.float32)
    b_out_f = np.asarray(b_out, dtype=np.float32)
    in_maps = _make_in_maps(x, w_qkv, w_out, b_out)

    from concourse.bass_utils import run_bass_kernel_spmd

    nc = _get_nc()
    res = run_bass_kernel_spmd(nc, in_maps, core_ids=list(range(NCORES))).results

    out = np.empty((B, N, DIM), dtype=np.float32)
    for c in range(NCORES):
        L = ENDS[c] - STARTS[c]
        out[:, STARTS[c]:ENDS[c], :] = res[c]["out_tokens"][:, :L, :]

    # CLS + tok1 rows from partial softmax stats
    for b in range(B):
        acc = np.zeros((HEADS, 769), dtype=np.float64)
        for c in range(NCORES):
            acc += res[c]["cls_part"][b].astype(np.float64)
            # padding keys on core c contributed exp(0)=1 to each head's sum
            acc[:, 768] -= 256 - (ENDS[c] - STARTS[c])
        t1x = res[0]["t1x_part"][b].astype(np.float64)  # [24, 769]
        acc += t1x[0::2]  # rows 2h: CLS query vs keys {CLS, tok1}
        cls_flat = np.empty(INNER, dtype=np.float64)
        t1_flat = np.empty(INNER, dtype=np.float64)
        for h in range(HEADS):
            cls_flat[64 * h:64 * h + 64] = acc[h, 64 * h:64 * h + 64] / acc[h, 768]
            t1_flat[64 * h:64 * h + 64] = t1x[2 * h + 1, 64 * h:64 * h + 64] / t1x[2 * h + 1, 768]
        out[b, 0, :] = (cls_flat @ w_out_f + b_out_f).astype(np.float32)
        out[b, 1, :] = (t1_flat @ w_out_f + b_out_f).astype(np.float32)
    return out


# revision 17
# speedup vs baseline: 3.9731x; 1.1806x over previous
import os
import sys

for _p in ("/opt/trn_rl_repo", "/root/.axon_site/_ro/trn_rl_repo"):
    if os.path.isdir(_p) and _p not in sys.path:
        sys.path.insert(0, _p)

import numpy as np
import ml_dtypes

BF16 = ml_dtypes.bfloat16

HEADS, D = 12, 64
WINDOW, SHIFT = 16, 1
SCALE = D ** -0.5
B, N, DIM = 2, 2049, 768
INNER = HEADS * D  # 768
TAUG = 258  # CLS slot + tok1/dummy slot + 256 block tokens
NCORES = 8
KT = DIM // 128  # 6
VW = HEADS * 65  # 780: per-head 64 v-cols + ones-col at 65h+64

STARTS = [2, 258, 514, 770, 1026, 1282, 1538, 1794]
ENDS = [258, 514, 770, 1026, 1282, 1538, 1794, 2049]

_NC_CACHE = {}


def _build_nc():
    import concourse.bass as bass
    import concourse.bacc as bacc
    import concourse.mybir as mybir
    import concourse.tile as tile

    f32 = mybir.dt.float32
    bf16 = mybir.dt.bfloat16
    Exp = mybir.ActivationFunctionType.Exp
    Copy = mybir.ActivationFunctionType.Copy

    nc = bacc.Bacc(None, target_bir_lowering=False)

    xT_ext = nc.declare_dram_parameter("xaT", (B, KT, 128, TAUG), bf16, isOutput=False)
    wqkv_ext = nc.declare_dram_parameter("w_qkv", (DIM, 1536 + VW), bf16, isOutput=False)
    wout_ext = nc.declare_dram_parameter("w_out", (INNER, DIM), bf16, isOutput=False)
    bout_ext = nc.declare_dram_parameter("b_out", (128, DIM), f32, isOutput=False)
    mask_ext = nc.declare_dram_parameter("masks", (2, 128, 128), bf16, isOutput=False)
    id_ext = nc.declare_dram_parameter("ident", (128, 128), bf16, isOutput=False)
    vcr_ext = nc.declare_dram_parameter("vc_rep", (B, 2, VW), bf16, isOutput=False)
    out_ext = nc.declare_dram_parameter("out_tokens", (B, 256, DIM), f32, isOutput=True)
    cls_ext = nc.declare_dram_parameter("cls_part", (B, HEADS, VW), f32, isOutput=True)
    t1x_ext = nc.declare_dram_parameter("t1x_part", (B, 2 * HEADS, VW), f32, isOutput=True)

    with tile.TileContext(nc) as tc:
        with (
            tc.tile_pool(name="wpool", bufs=1) as wp,
            tc.tile_pool(name="fpool", bufs=2) as fp,
            tc.tile_pool(name="spool", bufs=6) as sp,
            tc.tile_pool(name="psA", bufs=2, space="PSUM") as psA,
            tc.tile_pool(name="psP", bufs=2, space="PSUM") as psP,
            tc.tile_pool(name="psS", bufs=3, space="PSUM") as psS,
        ):
            ident = wp.tile([128, 128], bf16, tag="ident")
            nc.sync.dma_start(ident[:], id_ext[:])
            mask_t = []
            for s in range(2):
                m = wp.tile([128, 128], bf16, tag=f"mask{s}")
                nc.sync.dma_start(m[:], mask_ext[s])
                mask_t.append(m)
            w_tiles = []
            for k in range(KT):
                t = wp.tile([128, 1536 + VW], bf16, tag=f"wqkv{k}")
                nc.sync.dma_start(t[:], wqkv_ext[k * 128:(k + 1) * 128, :])
                w_tiles.append(t)

            xT, vcr = [], []
            for b in range(B):
                row = []
                for k in range(KT):
                    t = fp.tile([128, TAUG], bf16, tag=f"xT{b}_{k}", name=f"xT{b}_{k}")
                    nc.sync.dma_start(t[:], xT_ext[b, k])
                    row.append(t)
                xT.append(row)
                vt = fp.tile([2, VW], bf16, tag=f"vcr{b}", name=f"vcr{b}")
                nc.sync.dma_start(vt[:], vcr_ext[b])
                vcr.append(vt)

            wo_t = []
            for k in range(KT):
                t = wp.tile([128, DIM], bf16, tag=f"wo{k}")
                nc.sync.dma_start(t[:], wout_ext[k * 128:(k + 1) * 128, :])
                wo_t.append(t)
            bias_full = wp.tile([128, DIM], f32, tag="bias_full")
            nc.sync.dma_start(bias_full[:], bout_ext[:])

            # ---- QKV projections, batch-interleaved ----
            qT = [[None] * (HEADS // 2) for _ in range(B)]
            kTt = [[None] * (HEADS // 2) for _ in range(B)]
            for base, pref, dst in ((0, "q", qT), (INNER, "k", kTt)):
                for g in range(HEADS // 2):
                    for b in range(B):
                        ps = psA.tile([128, 512], f32, tag="big", name="ps")
                        for k in range(KT):
                            nc.tensor.matmul(ps[:, 0:TAUG],
                                             w_tiles[k][:, base + 128 * g: base + 128 * (g + 1)],
                                             xT[b][k][:], start=(k == 0), stop=(k == KT - 1))
                        t = fp.tile([128, TAUG], bf16, tag=f"{pref}T{b}_{g}", name=f"{pref}T{b}_{g}")
                        nc.vector.tensor_copy(t[:], ps[:, 0:TAUG])
                        dst[b][g] = t
            vs = [[None, None] for _ in range(B)]
            for ti in range(2):
                for b in range(B):
                    vt = fp.tile([128, VW], bf16, tag=f"v{b}_{ti}", name=f"v{b}_{ti}")
                    for c0, cw in ((0, 512), (512, VW - 512)):
                        pv = psA.tile([128, 512], f32, tag="big", name="pv")
                        for k in range(KT):
                            nc.tensor.matmul(pv[:, 0:cw],
                                             xT[b][k][:, 2 + 128 * ti: 2 + 128 * (ti + 1)],
                                             w_tiles[k][:, 1536 + c0: 1536 + c0 + cw],
                                             start=(k == 0), stop=(k == KT - 1))
                        nc.vector.tensor_copy(vt[:, c0:c0 + cw], pv[:, 0:cw])
                    nc.vector.memset(vt[:, 64:VW:65], 1.0)
                    vs[b][ti] = vt

            # ---- attention, batch-interleaved per head ----
            att_s = [[fp.tile([128, INNER], bf16, tag=f"att{b}_{s}", name=f"att{b}_{s}")
                      for s in range(2)] for b in range(B)]
            clspt = psS.tile([128, 512], f32, tag="cls", bufs=1, name="clspt")
            clsp = [clspt[:, 256 * b:256 * b + 256] for b in range(B)]
            for h in range(HEADS):
                g, p0 = h // 2, 64 * (h % 2)
                for b in range(B):
                    kk, qq = kTt[b][g], qT[b][g]
                    hps = psS.tile([128, 512], f32, tag="hps", name="hps")
                    nc.tensor.matmul(hps[0:1, 0:TAUG], kk[p0:p0 + 64, 0:1], qq[p0:p0 + 64, :],
                                     start=True, stop=True, skip_group_check=True)
                    ecr = sp.tile([1, TAUG], bf16, tag="ecr", name="ecr")
                    nc.scalar.activation(ecr[:], hps[0:1, 0:TAUG], Exp, scale=SCALE)
                    for s in range(2):
                        q0 = 2 + 128 * s
                        nc.tensor.matmul(clsp[b][:, 12 * s + h:12 * s + h + 1],
                                         kk[p0:p0 + 64, q0:q0 + 128],
                                         qq[p0:p0 + 64, 0:1], start=True, stop=True,
                                         skip_group_check=True)
                    nc.tensor.matmul(clsp[b][0:2, 24 + 2 * h:24 + 2 * h + 2],
                                     kk[p0:p0 + 64, 0:2],
                                     qq[p0:p0 + 64, 0:2], start=True, stop=True,
                                     skip_group_check=True)
                    for s in range(2):
                        q0 = 2 + 128 * s
                        pc = TAUG + 65 * s
                        pst = psP.tile([128, 128], f32, tag="pq", name="pst")
                        nc.tensor.matmul(pst[:], kk[p0:p0 + 64, q0:q0 + 128],
                                         qq[p0:p0 + 64, q0:q0 + 128], start=True, stop=True)
                        prob = sp.tile([128, 128], bf16, tag="prob", name="prob")
                        nc.scalar.activation(prob[:], pst[:], Exp, scale=SCALE)
                        nc.vector.tensor_mul(prob[:], prob[:], mask_t[s][:])
                        nc.tensor.matmul(hps[:, pc:pc + 65], prob[:],
                                         vs[b][s][:, 65 * h:65 * h + 65],
                                         start=True, stop=False, skip_group_check=True)
                        nc.tensor.matmul(hps[:, pc:pc + 65], ecr[:, q0:q0 + 128],
                                         vcr[b][0:1, 65 * h:65 * h + 65],
                                         start=False, stop=True, skip_group_check=True)
                        rec = sp.tile([128, 1], f32, tag="rec", name="rec")
                        nc.vector.reciprocal(rec[:], hps[:, pc + 64:pc + 65])
                        nc.scalar.activation(att_s[b][s][:, 64 * h:64 * h + 64],
                                             hps[:, pc:pc + 64], Copy, scale=rec[:, 0:1])

            # ---- CLS-query / tok1 partials per batch ----
            for b in range(B):
                eccs = []
                for s in range(2):
                    E = sp.tile([128, HEADS], bf16, tag="ECC", name="E")
                    nc.scalar.activation(E[:], clsp[b][:, 12 * s:12 * (s + 1)], Exp, scale=SCALE)
                    eccs.append(E)
                clsA = psS.tile([128, 512], f32, tag="hps", name="clsA")
                clsB = psS.tile([128, 512], f32, tag="hps", name="clsB")
                for s in range(2):
                    nc.tensor.matmul(clsA[0:HEADS, :], eccs[s][:], vs[b][s][:, 0:512],
                                     start=(s == 0), stop=(s == 1), skip_group_check=True)
                    nc.tensor.matmul(clsB[0:HEADS, 0:VW - 512], eccs[s][:], vs[b][s][:, 512:VW],
                                     start=(s == 0), stop=(s == 1), skip_group_check=True)
                cls_sb = sp.tile([HEADS, VW], f32, tag="clssb", name="cls_sb")
                nc.vector.tensor_copy(cls_sb[:, 0:512], clsA[0:HEADS, :])
                nc.vector.tensor_copy(cls_sb[:, 512:VW], clsB[0:HEADS, 0:VW - 512])
                nc.sync.dma_start(cls_ext[b], cls_sb[:])

                ET1 = sp.tile([2, 2 * HEADS], bf16, tag="ET1", name="ET1")
                nc.scalar.activation(ET1[:], clsp[b][0:2, 24:24 + 2 * HEADS], Exp, scale=SCALE)
                t1A = psS.tile([128, 512], f32, tag="hps", name="t1A")
                t1B = psS.tile([128, 512], f32, tag="hps", name="t1B")
                nc.tensor.matmul(t1A[0:24, :], ET1[:], vcr[b][0:2, 0:512], start=True, stop=True,
                                 skip_group_check=True)
                nc.tensor.matmul(t1B[0:24, 0:VW - 512], ET1[:], vcr[b][0:2, 512:VW], start=True,
                                 stop=True, skip_group_check=True)
                t1_sb = sp.tile([24, VW], f32, tag="t1sb", name="t1_sb")
                nc.vector.tensor_copy(t1_sb[:, 0:512], t1A[0:24, :])
                nc.vector.tensor_copy(t1_sb[:, 512:VW], t1B[0:24, 0:VW - 512])
                nc.sync.dma_start(t1x_ext[b], t1_sb[:])

            # ---- transposes + out projection, batch-interleaved ----
            aT = [[fp.tile([128, 256], bf16, tag=f"aT{b}_{i}", name=f"aT{b}_{i}")
                   for i in range(KT)] for b in range(B)]
            for i in range(KT):
                for b in range(B):
                    for s in range(2):
                        pt = psP.tile([128, 128], bf16, tag="pq", name="pt")
                        nc.tensor.transpose(pt[:], att_s[b][s][:, 128 * i:128 * (i + 1)], ident[:])
                        nc.vector.tensor_copy(aT[b][i][:, 128 * s:128 * (s + 1)], pt[:])
            for ti in range(2):
                for b in range(B):
                    ot = fp.tile([128, DIM], f32, tag=f"ot{b}", name=f"ot{b}")
                    for c0, cw in ((0, 512), (512, 256)):
                        po = psA.tile([128, 512], f32, tag="big", name="po")
                        for i in range(KT):
                            nc.tensor.matmul(po[:, 0:cw], aT[b][i][:, 128 * ti:128 * (ti + 1)],
                                             wo_t[i][:, c0:c0 + cw],
                                             start=(i == 0), stop=(i == KT - 1))
                        nc.vector.tensor_add(ot[:, c0:c0 + cw], po[:, 0:cw],
                                             bias_full[:, c0:c0 + cw])
                    nc.sync.dma_start(out_ext[b, 128 * ti:128 * (ti + 1), :], ot[:])

    nc.compile()
    return nc


def _get_nc():
    if "nc" not in _NC_CACHE:
        _NC_CACHE["nc"] = _build_nc()
    return _NC_CACHE["nc"]


def _make_masks(core):
    start = STARTS[core]
    masks = np.zeros((2, 128, 128), dtype=np.float32)
    for s in range(2):
        g = start + s * 128 + np.arange(128)
        real = g < 2049
        blk = (g - 2) // 16
        same = (blk[:, None] == blk[None, :]) & real[:, None] & real[None, :]
        masks[s] = same.astype(np.float32)
    return masks.astype(BF16)


def _make_in_maps(x, w_qkv, w_out, b_out):
    x = np.asarray(x, dtype=np.float32)
    w_qkv = np.asarray(w_qkv, dtype=np.float32)
    w_out_b = np.asarray(w_out, dtype=np.float32).astype(BF16)
    b_out = np.asarray(b_out, dtype=np.float32)

    w_dev = np.zeros((DIM, 1536 + VW), dtype=np.float32)
    w_dev[:, 0:1536] = w_qkv[:, 0:1536]
    for h in range(HEADS):
        w_dev[:, 1536 + 65 * h:1536 + 65 * h + 64] = w_qkv[:, 1536 + 64 * h:1536 + 64 * h + 64]
    w_qkv_b = w_dev.astype(BF16)

    w_v = w_qkv[:, 1536:]
    vcls = x[:, 0, :] @ w_v
    vtok1 = x[:, 1, :] @ w_v

    def v65(row768):
        out = np.zeros(VW, dtype=np.float32)
        for h in range(HEADS):
            out[65 * h:65 * h + 64] = row768[64 * h:64 * h + 64]
            out[65 * h + 64] = 1.0
        return out

    ident = np.eye(128, dtype=BF16)
    bias_tiled = np.tile(b_out.reshape(1, DIM), (128, 1)).astype(np.float32)
    in_maps = []
    for c in range(NCORES):
        xa = np.zeros((B, TAUG, DIM), dtype=np.float32)
        xa[:, 0, :] = x[:, 0, :]
        if c == 0:
            xa[:, 1, :] = x[:, 1, :]
        L = ENDS[c] - STARTS[c]
        xa[:, 2:2 + L, :] = x[:, STARTS[c]:ENDS[c], :]
        xaT = xa.transpose(0, 2, 1).reshape(B, KT, 128, TAUG).astype(BF16)
        vcr = np.zeros((B, 2, VW), dtype=np.float32)
        for b in range(B):
            vcr[b, 0] = v65(vcls[b])
            vcr[b, 1] = v65(vtok1[b] if c == 0 else np.zeros(INNER, np.float32))
        in_maps.append({
            "vc_rep": vcr.astype(BF16),
            "xaT": xaT,
            "w_qkv": w_qkv_b,
            "w_out": w_out_b,
            "b_out": bias_tiled,
            "masks": _make_masks(c),
            "ident": ident,
        })
    return in_maps


def kernel(x, w_qkv, w_out, b_out):
    w_out_f = np.asarray(w_out, dtype=np.float32)
    b_out_f = np.asarray(b_out, dtype=np.float32)
    in_maps = _make_in_maps(x, w_qkv, w_out, b_out)

    from concourse.bass_utils import run_bass_kernel_spmd

    nc = _get_nc()
    res = run_bass_kernel_spmd(nc, in_maps, core_ids=list(range(NCORES))).results

    out = np.empty((B, N, DIM), dtype=np.float32)
    for c in range(NCORES):
        L = ENDS[c] - STARTS[c]
        out[:, STARTS[c]:ENDS[c], :] = res[c]["out_tokens"][:, :L, :]

    for b in range(B):
        acc = np.zeros((HEADS, VW), dtype=np.float64)
        for c in range(NCORES):
            acc += res[c]["cls_part"][b].astype(np.float64)
            acc[:, 64::65] -= 256 - (ENDS[c] - STARTS[c])
        t1x = res[0]["t1x_part"][b].astype(np.float64)
        acc += t1x[0::2]
        cls_flat = np.empty(INNER, dtype=np.float64)
        t1_flat = np.empty(INNER, dtype=np.float64)
        for h in range(HEADS):
            cls_flat[64 * h:64 * h + 64] = acc[h, 65 * h:65 * h + 64] / acc[h, 65 * h + 64]
            t1_flat[64 * h:64 * h + 64] = (t1x[2 * h + 1, 65 * h:65 * h + 64]
                                           / t1x[2 * h + 1, 65 * h + 64])
        out[b, 0, :] = (cls_flat @ w_out_f + b_out_f).astype(np.float32)
        out[b, 1, :] = (t1_flat @ w_out_f + b_out_f).astype(np.float32)
    return out
